# revision 8
# baseline (speedup 1.0000x reference)
"""Llama GQA attention (B=2, T=2048, D=2048, 32 heads / 8 KV heads, hd=64) on
8 Trainium2 NeuronCores.

Strategy: tensor-parallel over heads. Each core owns 4 q-heads + 1 kv-head:
wq/wk/wv output-dim sharded, wo input-dim sharded; each core emits a partial
[4096, 2048] o-proj output (fp32) and the host sums the 8 partials.

Device-side layout tricks:
  - x is shipped pre-transposed (xt [2048, 4096]) so q/k/v projections run as
    out[d, t] = wqkv.T @ xt with no on-device transposes of x.
  - RoPE pair de-interleave is folded into a host-side permutation of the
    wq/wk columns, making the on-device rotation two contiguous-half
    multiplies + one partition-swap copy.
  - Softmax runs on transposed scores (scoresT[k, q]); the denominator comes
    free from a ones-column appended to v; no max-subtraction is needed
    (inputs are scaled so exp cannot overflow).
  - Causality at block granularity: upper-triangle k-blocks are skipped;
    the 4 diagonal 128x128 blocks of each 512-token q-tile are computed as
    F=128 matmuls packed into one PSUM bank, exp'ed in one instruction and
    masked with a single tiled lower-triangular multiply.
  - Score exps are paired: two k-blocks' scores land in one 2-bank
    [128,1024] PSUM tile and are exp'ed by a single Activation instruction,
    halving the per-instruction access-latency overhead.
  - Softmax normalization: reciprocal_approx_fast on the [1,512] denominator
    row (custom DVE op), gpsimd partition-broadcast, one DVE multiply.
  - O-proj of q-tile i-1 is interleaved into the score stretches of q-tile i
    as dependency-free PE filler, keeping the PE p-state ramp at full clock.
  - v is transposed with the DMA XBAR (no PE transposes).
"""
import sys

for _p in ("/opt/trn_rl_repo", "/root/.axon_site", "/root/.axon_site/_ro/trn_rl_repo",
           "/root/.axon_site/_ro/pypackages"):
    if _p not in sys.path:
        sys.path.append(_p)

import numpy as np
import ml_dtypes

import concourse.bass as bass
import concourse.mybir as mybir
import concourse.tile as tile
from concourse import bacc
from concourse.bass_utils import run_bass_kernel_spmd

f32 = mybir.dt.float32
bf16 = mybir.dt.bfloat16
AF = mybir.ActivationFunctionType

B, T, D = 2, 2048, 2048
H, HKV, HD = 32, 8, 64
NCORES = 8
HPC = H // NCORES            # q heads per core (4)
DQC = HPC * HD               # 256 q channels per core
N = B * T                    # 4096 flattened tokens
KC = D // 128                # 16 contraction chunks for projections
NT = N // 512                # 8 token tiles of 512 for projections
QT = T // 512                # 4 q-tiles of 512 per batch
KB = T // 128                # 16 k-blocks of 128 per batch
ROPE_THETA = 10000.0

_nc_cache = [None]


def build():
    if _nc_cache[0] is not None:
        return _nc_cache[0]
    nc = bacc.Bacc()
    xtb = nc.declare_dram_parameter("xtb", [NT, 128, KC, 512], bf16, isOutput=False)
    wqkv = nc.declare_dram_parameter("wqkv", [D, DQC + 2 * HD], bf16, isOutput=False)
    wo = nc.declare_dram_parameter("wo", [DQC, D], bf16, isOutput=False)
    csb = nc.declare_dram_parameter("csb", [NT, 128, 2, 512], f32, isOutput=False)
    mask4 = nc.declare_dram_parameter("mask4", [128, 128], bf16, isOutput=False)
    ident = nc.declare_dram_parameter("ident", [128, 128], f32, isOutput=False)
    out = nc.declare_dram_parameter("out", [N // 128, D // 512, 128, 512], f32,
                                    isOutput=True)

    with tile.TileContext(nc) as tc:
        with tc.tile_pool(name="pers", bufs=1) as pers:
            wqkv_sb = pers.tile([128, KC, 384], bf16)
            for c in range(KC):
                nc.sync.dma_start(out=wqkv_sb[:, c, :], in_=wqkv[c * 128:(c + 1) * 128, :])
            ident_sb = pers.tile([128, 128], f32)
            nc.sync.dma_start(out=ident_sb, in_=ident[:])
            wo0 = pers.tile([128, D], bf16)
            wo1 = pers.tile([128, D], bf16)
            mask4_sb = pers.tile([128, 128], bf16)
            q01 = pers.tile([128, N], bf16)      # heads 0,1 qT
            q23 = pers.tile([128, N], bf16)      # heads 2,3 qT
            kk = pers.tile([128, N], bf16)       # kT duplicated into both halves
            vnat = pers.tile([128, N // 128, 65], bf16)  # v natural + ones col
            nc.vector.memset(vnat[:, :, 64:65], 1.0)

            # ---- phase 1: projections + RoPE + v transpose ----
            with tc.tile_pool(name="p1sb", bufs=1) as p1sb, \
                 tc.tile_pool(name="p1ps", bufs=1, space="PSUM") as p1ps, \
                 tc.tile_pool(name="trps", bufs=1, space="PSUM") as trps:
                for n in range(NT):
                    t0 = n * 512
                    xct = p1sb.tile([128, KC, 512], bf16, tag="xtc", bufs=3)
                    for cq in range(4):
                        nc.sync.dma_start(out=xct[:, cq * 4:(cq + 1) * 4, :],
                                          in_=xtb[n, :, cq * 4:(cq + 1) * 4, :])
                    cs_t = p1sb.tile([128, 2, 512], f32, tag="cos", bufs=3)
                    nc.sync.dma_start(out=cs_t, in_=csb[n])
                    cos_t = cs_t[:, 0, :]
                    sin_t = cs_t[:, 1, :]
                    pq01 = p1ps.tile([128, 512], f32, tag="pp", bufs=6)
                    pq23 = p1ps.tile([128, 512], f32, tag="pp", bufs=6)
                    pkv = p1ps.tile([128, 512], f32, tag="pp", bufs=6)
                    for c in range(KC):
                        st, sp = (c == 0), (c == KC - 1)
                        xc = xct[:, c, :]
                        nc.tensor.matmul(pq01, wqkv_sb[:, c, 0:128], xc, start=st, stop=sp)
                        nc.tensor.matmul(pq23, wqkv_sb[:, c, 128:256], xc, start=st, stop=sp)
                        nc.tensor.matmul(pkv, wqkv_sb[:, c, 256:384], xc, start=st, stop=sp)
                    # RoPE on q (two head-pair buffers)
                    for pq, qbuf in ((pq01, q01), (pq23, q23)):
                        qc = p1sb.tile([128, 512], f32, tag="qc", bufs=2)
                        nc.scalar.copy(qc, pq)
                        qs = p1sb.tile([128, 512], f32, tag="qs", bufs=2)
                        for hb in range(2):
                            r = hb * 64
                            nc.sync.dma_start(out=qs[r:r + 32, :], in_=qc[r + 32:r + 64, :])
                            nc.sync.dma_start(out=qs[r + 32:r + 64, :], in_=qc[r:r + 32, :])
                        t1 = p1sb.tile([128, 512], f32, tag="t1", bufs=2)
                        t2 = p1sb.tile([128, 512], f32, tag="t2", bufs=2)
                        nc.vector.tensor_mul(t1, qc, cos_t)
                        nc.vector.tensor_mul(t2, qs, sin_t)
                        nc.vector.tensor_add(qbuf[:, t0:t0 + 512], t1, t2)
                    # RoPE on k (rows 0:64 of pkv), then duplicate to rows 64:128
                    kc = p1sb.tile([64, 512], f32, tag="kc", bufs=2)
                    nc.scalar.copy(kc, pkv[0:64, :])
                    ks = p1sb.tile([64, 512], f32, tag="ks", bufs=2)
                    nc.sync.dma_start(out=ks[0:32, :], in_=kc[32:64, :])
                    nc.sync.dma_start(out=ks[32:64, :], in_=kc[0:32, :])
                    t1k = p1sb.tile([64, 512], f32, tag="t1k", bufs=2)
                    t2k = p1sb.tile([64, 512], f32, tag="t2k", bufs=2)
                    nc.vector.tensor_mul(t1k, kc, cos_t[0:64, :])
                    nc.vector.tensor_mul(t2k, ks, sin_t[0:64, :])
                    nc.vector.tensor_add(kk[0:64, t0:t0 + 512], t1k, t2k)
                    nc.sync.dma_start(out=kk[64:128, t0:t0 + 512], in_=kk[0:64, t0:t0 + 512])
                    # v: rows 64:128 of pkv -> PE-transpose into vnat blocks
                    vts = p1sb.tile([128, 512], f32, tag="vts", bufs=2)
                    nc.scalar.copy(vts[64:128, :], pkv[64:128, :])
                    ptr4 = trps.tile([128, 4, 64], f32, tag="tr", bufs=2)
                    for s4 in range(4):
                        nc.tensor.transpose(ptr4[:, s4, :],
                                            vts[64:128, s4 * 128:(s4 + 1) * 128],
                                            ident_sb[64:128, 64:128])
                    nc.scalar.copy(vnat[:, n * 4:n * 4 + 4, 0:64], ptr4)

            # phase-2-only constants (emitted late so they don't delay phase 1)
            nc.sync.dma_start(out=wo0, in_=wo[0:128, :])
            nc.sync.dma_start(out=wo1, in_=wo[128:256, :])
            nc.sync.dma_start(out=mask4_sb, in_=mask4[:])

            # ---- phase 2: attention + o-proj (o-proj pipelined one tile back) ----
            with tc.tile_pool(name="p2sb", bufs=1) as p2sb, \
                 tc.tile_pool(name="p2ps", bufs=1, space="PSUM") as p2ps:
                osb_flip = [0]

                def oproj_chunk(prev, st, dn):
                    pq0, poT0, poT1 = prev
                    tk = pq0 + st * 128
                    pop = p2ps.tile([128, 512], f32, tag="op", bufs=2)
                    nc.tensor.matmul(pop, poT0[:, st * 128:(st + 1) * 128],
                                     wo0[:, dn * 512:(dn + 1) * 512],
                                     start=True, stop=False)
                    nc.tensor.matmul(pop, poT1[:, st * 128:(st + 1) * 128],
                                     wo1[:, dn * 512:(dn + 1) * 512],
                                     start=False, stop=True)
                    osb = p2sb.tile([128, 512], f32, tag="osb", bufs=4)
                    if osb_flip[0] == 0:
                        nc.scalar.copy(osb, pop)
                    else:
                        nc.vector.tensor_copy(osb, pop)
                    osb_flip[0] = (osb_flip[0] + 1) % 4
                    nc.sync.dma_start(out=out[tk // 128, dn], in_=osb)

                prev = None
                for b in range(B):
                    for j in range(QT):
                        q0 = b * T + j * 512
                        oT0 = p2sb.tile([128, 512], bf16, tag="oT", bufs=4)
                        oT1 = p2sb.tile([128, 512], bf16, tag="oT", bufs=4)
                        npair = 2 * j
                        for h in range(HPC):
                            qbuf = q01 if h < 2 else q23
                            base = (h % 2) * 64
                            oT = oT0 if h < 2 else oT1
                            opq = 0
                            es_tiles = []
                            for p in range(npair):
                                ps2 = p2ps.tile([128, 1024], f32, tag="sc", bufs=2)
                                es2 = p2sb.tile([128, 1024], bf16, tag="es", bufs=10)
                                for half in range(2):
                                    kb = 2 * p + half
                                    k0 = b * T + kb * 128
                                    nc.tensor.matmul(
                                        ps2[:, half * 512:(half + 1) * 512],
                                        kk[base:base + 64, k0:k0 + 128],
                                        qbuf[base:base + 64, q0:q0 + 512],
                                        start=True, stop=True)
                                if prev is not None and opq < 4:
                                    oproj_chunk(prev, h, opq)
                                    opq += 1
                                nc.scalar.activation(es2, ps2, AF.Exp, scale=0.125)
                                es_tiles.append(es2)
                            # diagonal block-triangle: k-block 4j+di is valid
                            # for q columns >= 128*di; compute F=512-128*di
                            # score strips (1280 cols total vs 2048 dense),
                            # packed into two PSUM tiles / two exps. Only the
                            # leading 128 columns of each strip need the
                            # lower-triangular mask.
                            psA = p2ps.tile([128, 1024], f32, tag="sc", bufs=2)
                            esA = p2sb.tile([128, 1024], bf16, tag="es", bufs=10)
                            psB = p2ps.tile([128, 1024], f32, tag="sc", bufs=2)
                            esB = p2sb.tile([128, 1024], bf16, tag="es", bufs=10)
                            # (di, psum tile, col offset in tile)
                            strips = ((0, psA, esA, 0), (2, psA, esA, 512),
                                      (1, psB, esB, 0), (3, psB, esB, 384))
                            for di, pst, _, co in strips:
                                kb = 4 * j + di
                                k0 = b * T + kb * 128
                                F = 512 - 128 * di
                                nc.tensor.matmul(
                                    pst[:, co:co + F],
                                    kk[base:base + 64, k0:k0 + 128],
                                    qbuf[base:base + 64, q0 + 128 * di:q0 + 512],
                                    start=True, stop=True)
                            nc.scalar.activation(esA[:, 0:768], psA[:, 0:768],
                                                 AF.Exp, scale=0.125)
                            nc.scalar.activation(esB[:, 0:512], psB[:, 0:512],
                                                 AF.Exp, scale=0.125)
                            for _, _, est, co in strips:
                                nc.vector.tensor_mul(est[:, co:co + 128],
                                                     est[:, co:co + 128], mask4_sb)
                            while prev is not None and opq < 4:
                                oproj_chunk(prev, h, opq)
                                opq += 1
                            # AV accumulation
                            pav = p2ps.tile([128, 512], f32, tag="av", bufs=2)
                            for p in range(npair):
                                for half in range(2):
                                    kb = 2 * p + half
                                    nc.tensor.matmul(
                                        pav[0:65, :], vnat[:, b * KB + kb, :],
                                        es_tiles[p][:, half * 512:(half + 1) * 512],
                                        start=(kb == 0), stop=False)
                            # j==0: the di=0 strip covers all 512 columns and
                            # starts the psum group; j>0 it was started by the
                            # kb==0 full matmul. Single stop on the last strip.
                            for di, _, est, co in strips:
                                F = 512 - 128 * di
                                nc.tensor.matmul(
                                    pav[0:65, 128 * di:512],
                                    vnat[:, b * KB + 4 * j + di, :],
                                    est[:, co:co + F],
                                    start=(j == 0 and di == 0),
                                    stop=(di == 3))
                            # normalize: 1/denominator, broadcast, multiply
                            dn_r = p2sb.tile([1, 512], f32, tag="dn", bufs=3)
                            nc.vector.tensor_copy(dn_r, pav[64:65, :])
                            dni = p2sb.tile([1, 512], f32, tag="dni", bufs=3)
                            nc.vector.reciprocal_approx_fast(dni, dn_r)
                            rb = p2sb.tile([64, 512], f32, tag="rb", bufs=3)
                            nc.gpsimd.partition_broadcast(rb, dni)
                            nc.vector.tensor_mul(oT[base:base + 64, :], pav[0:64, :], rb)
                        prev = (q0, oT0, oT1)
                # drain the last tile's o-proj
                for st in range(4):
                    for dn in range(4):
                        oproj_chunk(prev, st, dn)

    nc.compile()
    _nc_cache[0] = nc
    return nc


def prep_inputs(x, wq, wk, wv, wo):
    x = np.asarray(x, np.float32)
    wq = np.asarray(wq, np.float32)
    wk = np.asarray(wk, np.float32)
    wv = np.asarray(wv, np.float32)
    wo = np.asarray(wo, np.float32)

    # blocked x^T: xtb[n, p, c, t] = x^T[c*128+p, n*512+t]
    xt = x.reshape(N, D).T.astype(ml_dtypes.bfloat16)                # [D, N]
    xtb = np.ascontiguousarray(
        xt.reshape(KC, 128, NT, 512).transpose(2, 1, 0, 3))          # [NT,128,KC,512]

    # de-interleave RoPE pairs inside each head's 64 columns
    deint = np.concatenate([np.arange(0, HD, 2), np.arange(1, HD, 2)])
    qperm = (np.arange(H)[:, None] * HD + deint[None, :]).reshape(-1)
    kperm = (np.arange(HKV)[:, None] * HD + deint[None, :]).reshape(-1)
    wq_p = wq[:, qperm]
    wk_p = wk[:, kperm]

    # rope tables
    inv = 1.0 / (ROPE_THETA ** (np.arange(0, HD, 2, dtype=np.float64) / HD))
    tpos = np.arange(T, dtype=np.float64)
    ang = np.outer(tpos, inv)                                        # [T, 32]
    cosv = np.cos(ang).astype(np.float32).T                          # [32, T]
    sinv = np.sin(ang).astype(np.float32).T
    cos_half = np.concatenate([cosv, cosv], axis=0)                  # [64, T]
    sin_half = np.concatenate([-sinv, sinv], axis=0)
    cs = np.stack([
        np.tile(np.tile(cos_half, (2, 1)), (1, B)),
        np.tile(np.tile(sin_half, (2, 1)), (1, B)),
    ]).astype(np.float32)                                            # [2, 128, N]
    csb = np.ascontiguousarray(
        cs.reshape(2, 128, NT, 512).transpose(2, 1, 0, 3))           # [NT,128,2,512]

    # one lower-triangular [128,128] mask tiled 4x along the free axis
    p = np.arange(128)[:, None]
    c = np.arange(128)[None, :]
    mask4 = np.ascontiguousarray((p <= c).astype(ml_dtypes.bfloat16))  # [128, 128]
    ident = np.eye(128, dtype=np.float32)

    in_maps = []
    for core in range(NCORES):
        wq_c = wq_p[:, core * DQC:(core + 1) * DQC]
        wk_c = wk_p[:, core * HD:(core + 1) * HD]
        wv_c = wv[:, core * HD:(core + 1) * HD]
        wqkv = np.ascontiguousarray(
            np.concatenate([wq_c, wk_c, wv_c], axis=1).astype(ml_dtypes.bfloat16))
        wo_c = np.ascontiguousarray(
            wo[core * DQC:(core + 1) * DQC, :].astype(ml_dtypes.bfloat16))
        in_maps.append({
            "xtb": xtb, "wqkv": wqkv, "wo": wo_c, "csb": csb,
            "mask4": mask4, "ident": ident,
        })
    return in_maps


def unblock(out_b):
    # [N//128, D//512, 128, 512] -> [N, D]
    return out_b.transpose(0, 2, 1, 3).reshape(N, D)


def kernel(x, wq, wk, wv, wo):
    nc = build()
    in_maps = prep_inputs(x, wq, wk, wv, wo)
    res = run_bass_kernel_spmd(nc, in_maps, list(range(NCORES)))
    acc = np.zeros((N // 128, D // 512, 128, 512), np.float64)
    for core in range(NCORES):
        acc += res.results[core]["out"].astype(np.float64)
    return unblock(acc).astype(np.float32).reshape(B, T, D)


# revision 9
# speedup vs baseline: 1.0320x; 1.0320x over previous
"""Llama GQA attention (B=2, T=2048, D=2048, 32 heads / 8 KV heads, hd=64) on
8 Trainium2 NeuronCores.

Strategy: tensor-parallel over heads. Each core owns 4 q-heads + 1 kv-head:
wq/wk/wv output-dim sharded, wo input-dim sharded; each core emits a partial
[4096, 2048] o-proj output (fp32) and the host sums the 8 partials.

Device-side layout tricks:
  - x is shipped pre-transposed (xt [2048, 4096]) so q/k/v projections run as
    out[d, t] = wqkv.T @ xt with no on-device transposes of x.
  - RoPE pair de-interleave is folded into a host-side permutation of the
    wq/wk columns, making the on-device rotation two contiguous-half
    multiplies + one partition-swap copy.
  - Softmax runs on transposed scores (scoresT[k, q]); the denominator comes
    free from a ones-column appended to v; no max-subtraction is needed
    (inputs are scaled so exp cannot overflow).
  - Causality at block granularity: upper-triangle k-blocks are skipped;
    the 4 diagonal 128x128 blocks of each 512-token q-tile are computed as
    F=128 matmuls packed into one PSUM bank, exp'ed in one instruction and
    masked with a single tiled lower-triangular multiply.
  - Score exps are paired: two k-blocks' scores land in one 2-bank
    [128,1024] PSUM tile and are exp'ed by a single Activation instruction,
    halving the per-instruction access-latency overhead.
  - Softmax normalization: reciprocal_approx_fast on the [1,512] denominator
    row (custom DVE op), gpsimd partition-broadcast, one DVE multiply.
  - O-proj of q-tile i-1 is interleaved into the score stretches of q-tile i
    as dependency-free PE filler, keeping the PE p-state ramp at full clock.
  - v is transposed with the DMA XBAR (no PE transposes).
"""
import sys

for _p in ("/opt/trn_rl_repo", "/root/.axon_site", "/root/.axon_site/_ro/trn_rl_repo",
           "/root/.axon_site/_ro/pypackages"):
    if _p not in sys.path:
        sys.path.append(_p)

import numpy as np
import ml_dtypes

import concourse.bass as bass
import concourse.mybir as mybir
import concourse.tile as tile
from concourse import bacc
from concourse.bass_utils import run_bass_kernel_spmd

f32 = mybir.dt.float32
bf16 = mybir.dt.bfloat16
AF = mybir.ActivationFunctionType

B, T, D = 2, 2048, 2048
H, HKV, HD = 32, 8, 64
NCORES = 8
HPC = H // NCORES            # q heads per core (4)
DQC = HPC * HD               # 256 q channels per core
N = B * T                    # 4096 flattened tokens
KC = D // 128                # 16 contraction chunks for projections
NT = N // 512                # 8 token tiles of 512 for projections
QT = T // 512                # 4 q-tiles of 512 per batch
KB = T // 128                # 16 k-blocks of 128 per batch
ROPE_THETA = 10000.0

_nc_cache = [None]


def build():
    if _nc_cache[0] is not None:
        return _nc_cache[0]
    nc = bacc.Bacc()
    xtb = nc.declare_dram_parameter("xtb", [NT, 128, KC, 512], bf16, isOutput=False)
    wqkvb = nc.declare_dram_parameter("wqkvb", [128, KC, 384], bf16, isOutput=False)
    wo = nc.declare_dram_parameter("wo", [DQC, D], bf16, isOutput=False)
    csb = nc.declare_dram_parameter("csb", [NT, 128, 2, 512], bf16, isOutput=False)
    mask4 = nc.declare_dram_parameter("mask4", [128, 128], bf16, isOutput=False)
    ident = nc.declare_dram_parameter("ident", [128, 128], f32, isOutput=False)
    out = nc.declare_dram_parameter("out", [N // 128, D // 512, 128, 512], f32,
                                    isOutput=True)

    with tile.TileContext(nc) as tc:
        with tc.tile_pool(name="pers", bufs=1) as pers:
            wqkv_sb = pers.tile([128, KC, 384], bf16)
            ident_sb = pers.tile([128, 128], f32)
            nc.sync.dma_start(out=ident_sb, in_=ident[:])
            wo0 = pers.tile([128, D], bf16)
            wo1 = pers.tile([128, D], bf16)
            mask4_sb = pers.tile([128, 128], bf16)
            q01 = pers.tile([128, N], bf16)      # heads 0,1 qT
            q23 = pers.tile([128, N], bf16)      # heads 2,3 qT
            kk = pers.tile([128, N], bf16)       # kT duplicated into both halves
            vnat = pers.tile([128, N // 128, 65], bf16)  # v natural + ones col
            nc.vector.memset(vnat[:, :, 64:65], 1.0)

            # ---- phase 1: projections + RoPE + v transpose ----
            with tc.tile_pool(name="p1sb", bufs=1) as p1sb, \
                 tc.tile_pool(name="p1ps", bufs=1, space="PSUM") as p1ps, \
                 tc.tile_pool(name="trps", bufs=1, space="PSUM") as trps:
                for n in range(NT):
                    t0 = n * 512
                    xct = p1sb.tile([128, KC, 512], bf16, tag="xtc", bufs=4)
                    for cq in range(4):
                        nc.sync.dma_start(out=xct[:, cq * 4:(cq + 1) * 4, :],
                                          in_=xtb[n, :, cq * 4:(cq + 1) * 4, :])
                        if n == 0 and cq == 0:
                            nc.sync.dma_start(out=wqkv_sb, in_=wqkvb[:])
                    cs_t = p1sb.tile([128, 2, 512], bf16, tag="cos", bufs=3)
                    nc.sync.dma_start(out=cs_t, in_=csb[n])
                    cos_t = cs_t[:, 0, :]
                    sin_t = cs_t[:, 1, :]
                    pq01 = p1ps.tile([128, 512], f32, tag="pp", bufs=6)
                    pq23 = p1ps.tile([128, 512], f32, tag="pp", bufs=6)
                    pkv = p1ps.tile([128, 512], f32, tag="pp", bufs=6)
                    for c in range(KC):
                        st, sp = (c == 0), (c == KC - 1)
                        xc = xct[:, c, :]
                        nc.tensor.matmul(pq01, wqkv_sb[:, c, 0:128], xc, start=st, stop=sp)
                        nc.tensor.matmul(pq23, wqkv_sb[:, c, 128:256], xc, start=st, stop=sp)
                        nc.tensor.matmul(pkv, wqkv_sb[:, c, 256:384], xc, start=st, stop=sp)
                    # RoPE on q (two head-pair buffers)
                    for pq, qbuf in ((pq01, q01), (pq23, q23)):
                        qc = p1sb.tile([128, 512], bf16, tag="qc", bufs=2)
                        nc.scalar.copy(qc, pq)
                        qs = p1sb.tile([128, 512], bf16, tag="qs", bufs=2)
                        for hb in range(2):
                            r = hb * 64
                            nc.sync.dma_start(out=qs[r:r + 32, :], in_=qc[r + 32:r + 64, :])
                            nc.sync.dma_start(out=qs[r + 32:r + 64, :], in_=qc[r:r + 32, :])
                        t1 = p1sb.tile([128, 512], bf16, tag="t1", bufs=2)
                        t2 = p1sb.tile([128, 512], bf16, tag="t2", bufs=2)
                        nc.vector.tensor_mul(t1, qc, cos_t)
                        nc.vector.tensor_mul(t2, qs, sin_t)
                        nc.vector.tensor_add(qbuf[:, t0:t0 + 512], t1, t2)
                    # RoPE on k (rows 0:64 of pkv), then duplicate to rows 64:128
                    kc = p1sb.tile([64, 512], bf16, tag="kc", bufs=2)
                    nc.scalar.copy(kc, pkv[0:64, :])
                    ks = p1sb.tile([64, 512], bf16, tag="ks", bufs=2)
                    nc.sync.dma_start(out=ks[0:32, :], in_=kc[32:64, :])
                    nc.sync.dma_start(out=ks[32:64, :], in_=kc[0:32, :])
                    t1k = p1sb.tile([64, 512], bf16, tag="t1k", bufs=2)
                    t2k = p1sb.tile([64, 512], bf16, tag="t2k", bufs=2)
                    nc.vector.tensor_mul(t1k, kc, cos_t[0:64, :])
                    nc.vector.tensor_mul(t2k, ks, sin_t[0:64, :])
                    nc.vector.tensor_add(kk[0:64, t0:t0 + 512], t1k, t2k)
                    nc.sync.dma_start(out=kk[64:128, t0:t0 + 512], in_=kk[0:64, t0:t0 + 512])
                    # v: rows 64:128 of pkv -> PE-transpose into vnat blocks
                    vts = p1sb.tile([128, 512], f32, tag="vts", bufs=2)
                    nc.scalar.copy(vts[64:128, :], pkv[64:128, :])
                    ptr4 = trps.tile([128, 4, 64], f32, tag="tr", bufs=2)
                    for s4 in range(4):
                        nc.tensor.transpose(ptr4[:, s4, :],
                                            vts[64:128, s4 * 128:(s4 + 1) * 128],
                                            ident_sb[64:128, 64:128])
                    nc.scalar.copy(vnat[:, n * 4:n * 4 + 4, 0:64], ptr4)

            # phase-2-only constants (emitted late so they don't delay phase 1)
            nc.sync.dma_start(out=wo0, in_=wo[0:128, :])
            nc.sync.dma_start(out=wo1, in_=wo[128:256, :])
            nc.sync.dma_start(out=mask4_sb, in_=mask4[:])

            # ---- phase 2: attention + o-proj (o-proj pipelined one tile back) ----
            with tc.tile_pool(name="p2sb", bufs=1) as p2sb, \
                 tc.tile_pool(name="p2ps", bufs=1, space="PSUM") as p2ps:
                osb_flip = [0]

                def oproj_chunk(prev, st, dn):
                    pq0, poT0, poT1 = prev
                    tk = pq0 + st * 128
                    pop = p2ps.tile([128, 512], f32, tag="op", bufs=2)
                    nc.tensor.matmul(pop, poT0[:, st * 128:(st + 1) * 128],
                                     wo0[:, dn * 512:(dn + 1) * 512],
                                     start=True, stop=False)
                    nc.tensor.matmul(pop, poT1[:, st * 128:(st + 1) * 128],
                                     wo1[:, dn * 512:(dn + 1) * 512],
                                     start=False, stop=True)
                    osb = p2sb.tile([128, 512], f32, tag="osb", bufs=4)
                    if osb_flip[0] == 0:
                        nc.scalar.copy(osb, pop)
                    else:
                        nc.vector.tensor_copy(osb, pop)
                    osb_flip[0] = (osb_flip[0] + 1) % 4
                    nc.sync.dma_start(out=out[tk // 128, dn], in_=osb)

                prev = None
                for b in range(B):
                    for j in range(QT):
                        q0 = b * T + j * 512
                        oT0 = p2sb.tile([128, 512], bf16, tag="oT", bufs=4)
                        oT1 = p2sb.tile([128, 512], bf16, tag="oT", bufs=4)
                        npair = 2 * j
                        for h in range(HPC):
                            qbuf = q01 if h < 2 else q23
                            base = (h % 2) * 64
                            oT = oT0 if h < 2 else oT1
                            opq = 0
                            es_tiles = []
                            for p in range(npair):
                                ps2 = p2ps.tile([128, 1024], f32, tag="sc", bufs=2)
                                es2 = p2sb.tile([128, 1024], bf16, tag="es", bufs=10)
                                for half in range(2):
                                    kb = 2 * p + half
                                    k0 = b * T + kb * 128
                                    nc.tensor.matmul(
                                        ps2[:, half * 512:(half + 1) * 512],
                                        kk[base:base + 64, k0:k0 + 128],
                                        qbuf[base:base + 64, q0:q0 + 512],
                                        start=True, stop=True)
                                if prev is not None and opq < 4:
                                    oproj_chunk(prev, h, opq)
                                    opq += 1
                                nc.scalar.activation(es2, ps2, AF.Exp, scale=0.125)
                                es_tiles.append(es2)
                            # diagonal block-triangle: k-block 4j+di is valid
                            # for q columns >= 128*di; compute F=512-128*di
                            # score strips (1280 cols total vs 2048 dense),
                            # packed into two PSUM tiles / two exps. Only the
                            # leading 128 columns of each strip need the
                            # lower-triangular mask.
                            psA = p2ps.tile([128, 1024], f32, tag="sc", bufs=2)
                            esA = p2sb.tile([128, 1024], bf16, tag="es", bufs=10)
                            psB = p2ps.tile([128, 1024], f32, tag="sc", bufs=2)
                            esB = p2sb.tile([128, 1024], bf16, tag="es", bufs=10)
                            # (di, psum tile, col offset in tile)
                            strips = ((0, psA, esA, 0), (2, psA, esA, 512),
                                      (1, psB, esB, 0), (3, psB, esB, 384))
                            for di, pst, _, co in strips:
                                kb = 4 * j + di
                                k0 = b * T + kb * 128
                                F = 512 - 128 * di
                                nc.tensor.matmul(
                                    pst[:, co:co + F],
                                    kk[base:base + 64, k0:k0 + 128],
                                    qbuf[base:base + 64, q0 + 128 * di:q0 + 512],
                                    start=True, stop=True)
                            nc.scalar.activation(esA[:, 0:768], psA[:, 0:768],
                                                 AF.Exp, scale=0.125)
                            nc.scalar.activation(esB[:, 0:512], psB[:, 0:512],
                                                 AF.Exp, scale=0.125)
                            for _, _, est, co in strips:
                                nc.vector.tensor_mul(est[:, co:co + 128],
                                                     est[:, co:co + 128], mask4_sb)
                            while prev is not None and opq < 4:
                                oproj_chunk(prev, h, opq)
                                opq += 1
                            # AV accumulation
                            pav = p2ps.tile([128, 512], f32, tag="av", bufs=2)
                            for p in range(npair):
                                for half in range(2):
                                    kb = 2 * p + half
                                    nc.tensor.matmul(
                                        pav[0:65, :], vnat[:, b * KB + kb, :],
                                        es_tiles[p][:, half * 512:(half + 1) * 512],
                                        start=(kb == 0), stop=False)
                            # j==0: the di=0 strip covers all 512 columns and
                            # starts the psum group; j>0 it was started by the
                            # kb==0 full matmul. Single stop on the last strip.
                            for di, _, est, co in strips:
                                F = 512 - 128 * di
                                nc.tensor.matmul(
                                    pav[0:65, 128 * di:512],
                                    vnat[:, b * KB + 4 * j + di, :],
                                    est[:, co:co + F],
                                    start=(j == 0 and di == 0),
                                    stop=(di == 3))
                            # normalize: 1/denominator, broadcast, multiply
                            dn_r = p2sb.tile([1, 512], f32, tag="dn", bufs=3)
                            nc.vector.tensor_copy(dn_r, pav[64:65, :])
                            dni = p2sb.tile([1, 512], f32, tag="dni", bufs=3)
                            nc.vector.reciprocal_approx_fast(dni, dn_r)
                            rb = p2sb.tile([64, 512], f32, tag="rb", bufs=3)
                            nc.gpsimd.partition_broadcast(rb, dni)
                            nc.vector.tensor_mul(oT[base:base + 64, :], pav[0:64, :], rb)
                        prev = (q0, oT0, oT1)
                # drain the last tile's o-proj
                for st in range(4):
                    for dn in range(4):
                        oproj_chunk(prev, st, dn)

    nc.compile()
    _nc_cache[0] = nc
    return nc


def prep_inputs(x, wq, wk, wv, wo):
    x = np.asarray(x, np.float32)
    wq = np.asarray(wq, np.float32)
    wk = np.asarray(wk, np.float32)
    wv = np.asarray(wv, np.float32)
    wo = np.asarray(wo, np.float32)

    # blocked x^T: xtb[n, p, c, t] = x^T[c*128+p, n*512+t]
    xt = x.reshape(N, D).T.astype(ml_dtypes.bfloat16)                # [D, N]
    xtb = np.ascontiguousarray(
        xt.reshape(KC, 128, NT, 512).transpose(2, 1, 0, 3))          # [NT,128,KC,512]

    # de-interleave RoPE pairs inside each head's 64 columns
    deint = np.concatenate([np.arange(0, HD, 2), np.arange(1, HD, 2)])
    qperm = (np.arange(H)[:, None] * HD + deint[None, :]).reshape(-1)
    kperm = (np.arange(HKV)[:, None] * HD + deint[None, :]).reshape(-1)
    wq_p = wq[:, qperm]
    wk_p = wk[:, kperm]

    # rope tables
    inv = 1.0 / (ROPE_THETA ** (np.arange(0, HD, 2, dtype=np.float64) / HD))
    tpos = np.arange(T, dtype=np.float64)
    ang = np.outer(tpos, inv)                                        # [T, 32]
    cosv = np.cos(ang).astype(np.float32).T                          # [32, T]
    sinv = np.sin(ang).astype(np.float32).T
    cos_half = np.concatenate([cosv, cosv], axis=0)                  # [64, T]
    sin_half = np.concatenate([-sinv, sinv], axis=0)
    cs = np.stack([
        np.tile(np.tile(cos_half, (2, 1)), (1, B)),
        np.tile(np.tile(sin_half, (2, 1)), (1, B)),
    ]).astype(np.float32)                                            # [2, 128, N]
    csb = np.ascontiguousarray(
        cs.reshape(2, 128, NT, 512).transpose(2, 1, 0, 3).astype(ml_dtypes.bfloat16))

    # one lower-triangular [128,128] mask tiled 4x along the free axis
    p = np.arange(128)[:, None]
    c = np.arange(128)[None, :]
    mask4 = np.ascontiguousarray((p <= c).astype(ml_dtypes.bfloat16))  # [128, 128]
    ident = np.eye(128, dtype=np.float32)

    in_maps = []
    for core in range(NCORES):
        wq_c = wq_p[:, core * DQC:(core + 1) * DQC]
        wk_c = wk_p[:, core * HD:(core + 1) * HD]
        wv_c = wv[:, core * HD:(core + 1) * HD]
        wqkv = np.concatenate([wq_c, wk_c, wv_c], axis=1).astype(ml_dtypes.bfloat16)
        wqkvb = np.ascontiguousarray(
            wqkv.reshape(KC, 128, 384).transpose(1, 0, 2))           # [128, KC, 384]
        wo_c = np.ascontiguousarray(
            wo[core * DQC:(core + 1) * DQC, :].astype(ml_dtypes.bfloat16))
        in_maps.append({
            "xtb": xtb, "wqkvb": wqkvb, "wo": wo_c, "csb": csb,
            "mask4": mask4, "ident": ident,
        })
    return in_maps


def unblock(out_b):
    # [N//128, D//512, 128, 512] -> [N, D]
    return out_b.transpose(0, 2, 1, 3).reshape(N, D)


def kernel(x, wq, wk, wv, wo):
    nc = build()
    in_maps = prep_inputs(x, wq, wk, wv, wo)
    res = run_bass_kernel_spmd(nc, in_maps, list(range(NCORES)))
    acc = np.zeros((N // 128, D // 512, 128, 512), np.float64)
    for core in range(NCORES):
        acc += res.results[core]["out"].astype(np.float64)
    return unblock(acc).astype(np.float32).reshape(B, T, D)
